# revision 13
# baseline (speedup 1.0000x reference)
"""AttentionBlock (GroupNorm + MHSA + proj + residual) on 8 TRN2 NeuronCores.

Sharding: data-parallel over batch (B=8 -> 1 batch element per core), SPMD.

v3: software-pipelined attention. The scores->exp->av dependency chain is
decoupled by keeping each pair's exp(S) tiles (pt) in SBUF: pair p's slots
run scores(p) + av(p-1) + qkv-projection(p+1) back to back on the PE, with
exp (ScalarE exact for even heads / VectorE Schraudolph bf16 fast-exp for
odd heads) fully hidden underneath. PSUM banks: scores 2x2, av accumulators
2x1 (per (head, t-half) group), qk/zb 1x2. Softmax normalization and the
out-projection are interleaved with the last pair's av in a tail "pair".
Other key points vs the original baseline: bf16 weights/activations for
everything except the fp32 GroupNorm + residual, x-first DMA ordering,
PE warm-up matmuls under GroupNorm (HAM clock gate), rstd via
exp(-0.5*ln(var+eps)) so only one ACT table set is ever loaded, biases
folded into ScalarE copies / the residual instead of matmuls.
"""

import sys
import numpy as np

sys.path.insert(0, "/opt/trn_rl_repo")

import concourse.bacc as bacc
import concourse.bass as bass
import concourse.mybir as mybir
import concourse.tile as tile
from concourse import bass_utils

F32 = mybir.dt.float32
F32R = mybir.dt.float32r
BF16 = mybir.dt.bfloat16
I16 = mybir.dt.int16
AF = mybir.ActivationFunctionType
ALU = mybir.AluOpType

B, C, HH, WW = 8, 512, 32, 32
T = HH * WW            # 1024
NH = 8                 # heads
CH = C // NH           # 64 per-head dim
NCT = C // 128         # 4 channel tiles
NTT = T // 128         # 8 seq tiles
SCALE = 1.0 / np.sqrt(np.sqrt(CH))
EPS = 1e-5
VW = NH * (CH + 1)     # 520: v-section width incl per-head Z column

# Schraudolph fast-exp constants, bf16 flavor: i16 = rint(S * EA + EB)
# gives the int16 bit pattern of bf16(~exp(S)). Max multiplicative error
# ~ +-3.5% (centered); cancels partially in softmax.
EA = float((1 << 7) / np.log(2.0))
EB = float(127.0 * (1 << 7) - 0.5 * 0.0860713320559342 * (1 << 7) + 0.5)

_CACHE = {}


def build_kernel(debug=False):
    nc = bacc.Bacc(
        "TRN2", target_bir_lowering=False, debug=debug, num_devices=8
    )

    x_d = nc.dram_tensor("x", (C, T), F32, kind="ExternalInput")
    wqkvT_d = nc.dram_tensor("wqkvT", (C, 2 * C + VW), BF16, kind="ExternalInput")
    wprojT_d = nc.dram_tensor("wprojT", (C, C), BF16, kind="ExternalInput")
    cblob_d = nc.dram_tensor("cblob", (128, 2 * NCT + 8), F32, kind="ExternalInput")
    bcols_d = nc.dram_tensor("bcols", (128, 12), F32, kind="ExternalInput")
    vrow_d = nc.dram_tensor("vrow", (1, VW + 512), BF16, kind="ExternalInput")
    gbcast_d = nc.dram_tensor("gbcast", (8, 128), F32, kind="ExternalInput")
    e8_d = nc.dram_tensor("e8", (4, NH * CH), F32R, kind="ExternalInput")
    y_d = nc.dram_tensor("y", (C, T), F32, kind="ExternalOutput")

    with tile.TileContext(nc) as tc:
        with (
            tc.tile_pool(name="single", bufs=1) as single,
            tc.tile_pool(name="hp", bufs=NCT) as hp,
            tc.tile_pool(name="vt", bufs=NTT) as vtp,
            tc.tile_pool(name="qk", bufs=4) as qkp,
            tc.tile_pool(name="ptA", bufs=2 * NTT) as ptAp,
            tc.tile_pool(name="ptB", bufs=2 * NTT) as ptBp,
            tc.tile_pool(name="aun", bufs=1) as aunp,
            tc.tile_pool(name="aall", bufs=NCT) as aallp,
            tc.tile_pool(name="zp", bufs=1) as zp,
            tc.tile_pool(name="tmp", bufs=2) as tmpp,
            tc.tile_pool(name="gn", bufs=8) as gnp,
            tc.tile_pool(name="ps", bufs=1, space="PSUM") as pp,
        ):
            # PSUM tags: sc 2x[128,1024] (4 banks), av 2x[128,512] (2 banks),
            # qk 1x[128,1024] (2 banks) -> 8 banks exactly.
            def sc_tile(name):
                return pp.tile([128, T], F32, tag="sc", bufs=2, name=name)

            def av_tile(name):
                return pp.tile([128, 512], F32, tag="av", bufs=2, name=name)

            def qk_tile(name):
                return pp.tile([128, T], F32, tag="qk", bufs=1, name=name)

            # ---------------- x first, then consts, then weights ----------
            xbig = single.tile([128, NCT, T], F32, tag="xbig")
            xr4 = x_d.ap().rearrange("(c p) t -> p c t", p=128)
            for ct in range(NCT):
                nc.sync.dma_start(out=xbig[:, ct, :], in_=xr4[:, ct, :])
            x_t = [xbig[:, ct, :] for ct in range(NCT)]

            cblob = single.tile([128, 2 * NCT + 8], F32, tag="cblob")
            nc.sync.dma_start(out=cblob[:, :], in_=cblob_d.ap())
            gamma = cblob[:, 0:NCT]
            beta = cblob[:, NCT:2 * NCT]
            gred = cblob[:, 2 * NCT:2 * NCT + 8]
            bcols = single.tile([128, 12], F32, tag="bcols")
            nc.sync.dma_start(out=bcols[:, :], in_=bcols_d.ap())
            gbcast = single.tile([8, 128], F32, tag="gbcast")
            nc.sync.dma_start(out=gbcast[:, :], in_=gbcast_d.ap())
            e8 = single.tile([4, NH * CH], F32R, tag="e8")
            nc.sync.dma_start(out=e8[:, :], in_=e8_d.ap())
            vrow = single.tile([1, VW + 512], BF16, tag="vrow")
            nc.sync.dma_start(out=vrow[:, :], in_=vrow_d.ap())
            vbias = vrow[:, 0:VW]
            ones = vrow[:, VW:VW + 512]

            # preload the natural_log_exp ACT table set during DMA wait
            tldm = gnp.tile([1, 1], F32, tag="tld")
            nc.scalar.activation(out=tldm[:, :], in_=cblob[0:1, 0:1], func=AF.Ln)

            wqbig = single.tile([128, NCT, 2 * C + VW], BF16, tag="wqbig")
            wqr = wqkvT_d.ap().rearrange("(c p) t -> p c t", p=128)
            # v-section first (vT is computed first), then qk columns
            nc.sync.dma_start(
                out=wqbig[:, :, 2 * C:2 * C + VW], in_=wqr[:, :, 2 * C:2 * C + VW]
            )
            nc.sync.dma_start(out=wqbig[:, :, 0:2 * C], in_=wqr[:, :, 0:2 * C])
            wq_t = [wqbig[:, ct, :] for ct in range(NCT)]
            wpbig = single.tile([128, NCT, C], BF16, tag="wpbig")
            nc.sync.dma_start(
                out=wpbig[:, :, :],
                in_=wprojT_d.ap().rearrange("(c p) t -> p c t", p=128),
            )
            wp_t = [wpbig[:, ct, :] for ct in range(NCT)]

            # ---------------- PE warm-up (HAM clock gate) -----------------
            # ~5us of fp32 matmuls on x tile 0 flip the PE clock gate to
            # 2.4GHz before the real matmul stream begins.
            for w in range(3):
                wps = av_tile(f"warm{w}")
                nc.tensor.matmul(
                    wps[:, :], x_t[0][:, 0:128], x_t[0][:, 0:512],
                    start=True, stop=True,
                )

            # ---------------- GroupNorm ----------------
            cs = gnp.tile([128, 2 * NCT], F32, tag="cs")
            for ct in range(NCT):
                xr = x_t[ct][:, :].rearrange("p (n f) -> p n f", f=512)
                st = gnp.tile([128, 2, 6], F32, tag="st")
                for sg in range(2):
                    nc.vector.bn_stats(out=st[:, sg, :], in_=xr[:, sg, :])
                mv = gnp.tile([128, 2], F32, tag="mv")
                nc.vector.bn_aggr(out=mv[:, :], in_=st[:, :, :])
                nc.vector.tensor_copy(out=cs[:, ct:ct + 1], in_=mv[:, 0:1])
                nc.vector.tensor_mul(
                    out=cs[:, NCT + ct:NCT + ct + 1], in0=mv[:, 0:1], in1=mv[:, 0:1]
                )
                nc.vector.tensor_add(
                    out=cs[:, NCT + ct:NCT + ct + 1],
                    in0=cs[:, NCT + ct:NCT + ct + 1],
                    in1=mv[:, 1:2],
                )
            gsp = av_tile("gsp")
            nc.tensor.matmul(
                gsp[0:8, 0:2 * NCT], gred[:, :], cs[:, :], start=True, stop=True
            )
            gs = gnp.tile([8, 2 * NCT], F32, tag="gs")
            nc.vector.tensor_copy(out=gs[:, :], in_=gsp[0:8, 0:2 * NCT])
            # var = E[x^2] - mean^2; rstd = exp(-0.5*ln(var+eps))
            t1 = gnp.tile([8, NCT], F32, tag="t1")
            veps = gnp.tile([8, NCT], F32, tag="veps")
            nc.vector.tensor_mul(out=t1[:, :], in0=gs[:, 0:NCT], in1=gs[:, 0:NCT])
            nc.vector.tensor_sub(out=veps[:, :], in0=gs[:, NCT:], in1=t1[:, :])
            nc.vector.tensor_scalar_add(out=veps[:, :], in0=veps[:, :], scalar1=EPS)
            lv = gnp.tile([8, NCT], F32, tag="lv")
            nc.scalar.activation(out=lv[:, :], in_=veps[:, :], func=AF.Ln)
            r0 = gnp.tile([8, NCT], F32, tag="r0")
            nc.scalar.activation(out=r0[:, :], in_=lv[:, :], func=AF.Exp, scale=-0.5)
            mr = gnp.tile([8, 2 * NCT], F32, tag="mr")
            for ct in range(NCT):
                nc.vector.tensor_copy(
                    out=mr[:, 2 * ct:2 * ct + 1], in_=gs[:, ct:ct + 1]
                )
                nc.vector.tensor_copy(
                    out=mr[:, 2 * ct + 1:2 * ct + 2], in_=r0[:, ct:ct + 1]
                )
            h_t = []
            with nc.allow_low_precision(reason="bf16 matmul operands"):
                for ct in range(NCT):
                    mrc = av_tile(f"mrc{ct}")
                    nc.tensor.matmul(
                        mrc[:, 0:2], gbcast[:, :], mr[:, 2 * ct:2 * ct + 2],
                        start=True, stop=True,
                    )
                    sc_ = gnp.tile([128, 1], F32, tag="scg")
                    sh = gnp.tile([128, 1], F32, tag="shg")
                    nc.vector.tensor_mul(
                        out=sc_[:, :], in0=mrc[:, 1:2], in1=gamma[:, ct:ct + 1]
                    )
                    nc.vector.tensor_mul(out=sh[:, :], in0=mrc[:, 0:1], in1=sc_[:, :])
                    nc.vector.tensor_sub(
                        out=sh[:, :], in0=beta[:, ct:ct + 1], in1=sh[:, :]
                    )
                    ht = hp.tile([128, T], BF16, tag="h")
                    nc.scalar.activation(
                        out=ht[:, :], in_=x_t[ct][:, :], func=AF.Identity,
                        scale=sc_[:, :], bias=sh[:, :],
                    )
                    h_t.append(ht)

            # more warm-up matmuls: bridge the GroupNorm DVE latency so the
            # HAM MID window never sees the PE idle before vT starts
            for w in range(3, 9):
                wps = av_tile(f"warm{w}")
                nc.tensor.matmul(
                    wps[:, :], x_t[0][:, 0:128], x_t[0][:, 0:512],
                    start=True, stop=True,
                )

            # ---------------- v^T (+ per-head Z columns) ----------------
            vt_t = []
            with nc.allow_low_precision(reason="bf16 av operands"):
                for tt in range(NTT):
                    vps = sc_tile(f"vps{tt}")
                    for seg in ((0, 512), (512, VW)):
                        dst = vps[:, seg[0]:seg[1]]
                        for ct in range(NCT):
                            nc.tensor.matmul(
                                dst,
                                h_t[ct][:, tt * 128:(tt + 1) * 128],
                                wq_t[ct][:, 2 * C + seg[0]:2 * C + seg[1]],
                                start=(ct == 0), stop=False,
                            )
                        nc.tensor.matmul(
                            dst, ones[0:1, 0:128],
                            vbias[0:1, seg[0]:seg[1]],
                            start=False, stop=True,
                        )
                    vt = vtp.tile([128, VW], BF16, tag="vt")
                    nc.vector.tensor_copy(out=vt[:, :], in_=vps[:, 0:VW])
                    vt_t.append(vt)

            # fold proj bias into the residual: x += bproj (per-partition);
            # emitted here so it doesn't delay the GN->vT DVE chain
            for ct in range(NCT):
                nc.vector.tensor_scalar_add(
                    out=x_t[ct][:, :], in0=x_t[ct][:, :],
                    scalar1=bcols[:, 8 + ct:9 + ct],
                )

            # ------------- pipelined attention helpers -------------
            def emit_qk_mms(p, psum, half, nqs=(0, 1)):
                """half 0 -> q psum, 1 -> k psum; 4 matmuls per nq chunk."""
                off = 256 * p + 128 * half
                for nq in nqs:
                    dst = psum[:, nq * 512:(nq + 1) * 512]
                    for ct in range(NCT):
                        nc.tensor.matmul(
                            dst,
                            wq_t[ct][:, off:off + 128],
                            h_t[ct][:, nq * 512:(nq + 1) * 512],
                            start=(ct == 0), stop=(ct == NCT - 1),
                        )

            def emit_q_copy(p, psum):
                qs = qkp.tile([128, T], BF16, tag="qkpair", name=f"qp{p}")
                with nc.allow_low_precision(reason="bf16 matmul operands"):
                    nc.scalar.activation(
                        out=qs[:, :], in_=psum[:, :], func=AF.Identity,
                        bias=bcols[:, 2 * p:2 * p + 1],
                    )
                return qs

            def emit_k_copy(p, psum):
                ks = qkp.tile([128, T], BF16, tag="qkpair", name=f"kp{p}")
                with nc.allow_low_precision(reason="bf16 matmul operands"):
                    nc.vector.tensor_scalar_add(
                        out=ks[:, :], in0=psum[:, :],
                        scalar1=bcols[:, 2 * p + 1:2 * p + 2],
                    )
                return ks

            # state carried across pairs
            qp_s = kp_s = None          # current pair's q/k sbuf tiles
            qk_next_ps = None           # next pair's q or k psum in flight
            qp_next = None
            pts_prev = None             # previous pair's pt tiles
            av_group = [None] * 4       # live av psum tiles by group

            # qk(0) upfront: q via sc-tag ring (free now), k via qk-tag
            qps0 = sc_tile("qps0")
            emit_qk_mms(0, qps0, 0)
            kps0 = qk_tile("kps0")
            emit_qk_mms(0, kps0, 1)
            qp_s = emit_q_copy(0, qps0)
            kp_s = emit_k_copy(0, kps0)

            aunbig = aunp.tile([CH + 1, NH, T], F32, tag="aun")
            zallA = zp.tile([4, T], F32, tag="zA")
            zallB = zp.tile([4, T], F32, tag="zB")
            nc.vector.memset(zallB[:, :], 1.0)
            invzfA = zp.tile([4, T], F32, tag="izfA")
            invzfB = zp.tile([4, T], F32, tag="izfB")
            zscr = zp.tile([4, T], F32, tag="zscr")
            invzA = zp.tile([4, T], F32R, tag="izA")
            invzB = zp.tile([4, T], F32R, tag="izB")

            def emit_av_group_mms(p, g, pts):
                """All 8 accumulating av matmuls of group g=(h,nq) of pair p."""
                hl, nq = g // 2, g % 2
                h_ = 2 * p + hl
                avt = av_tile(f"av{p}_{g}")
                av_group[g] = avt
                for st_ in range(NTT):
                    nc.tensor.matmul(
                        avt[0:CH + 1, :],
                        vt_t[st_][:, h_ * (CH + 1):(h_ + 1) * (CH + 1)],
                        pts[st_][hl][:, nq * 512:(nq + 1) * 512],
                        start=(st_ == 0), stop=(st_ == NTT - 1),
                    )

            def emit_av_group_copy(p, g, engine):
                hl, nq = g // 2, g % 2
                h_ = 2 * p + hl
                avt = av_group[g]
                dst = aunbig[0:CH + 1, h_, nq * 512:(nq + 1) * 512]
                if engine == "act":
                    nc.scalar.activation(
                        out=dst, in_=avt[0:CH + 1, :], func=AF.Identity
                    )
                else:
                    nc.vector.tensor_copy(out=dst, in_=avt[0:CH + 1, :])

            # ---------------- pairs 0..3: scores + exp + av(p-1) ----------
            for p in range(4):
                pts_cur = []
                for st_ in range(NTT):
                    # scores for both heads (row-group packed, 2 concurrent)
                    scA = sc_tile(f"sc{p}_{st_}_0")
                    scB = sc_tile(f"sc{p}_{st_}_1")
                    scps = [scA, scB]
                    for nq in range(2):
                        for hl in range(2):
                            base = 64 * hl
                            nc.tensor.matmul(
                                scps[hl][:, nq * 512:(nq + 1) * 512],
                                kp_s[base:base + 64, st_ * 128:(st_ + 1) * 128],
                                qp_s[base:base + 64, nq * 512:(nq + 1) * 512],
                                start=True, stop=True,
                            )
                    # exp: even head exact on ScalarE; odd head Schraudolph.
                    # Emitted per 512-half so each scores bank frees as soon
                    # as its half is read (keeps next slot's scores unblocked).
                    with nc.allow_low_precision(reason="bf16 av operands"):
                        ptA = ptAp.tile([128, T], BF16, tag="ptA",
                                        name=f"ptA{p}_{st_}")
                        ptB = ptBp.tile([128, T], I16, tag="ptB",
                                        name=f"ptB{p}_{st_}")
                        for nq in range(2):
                            sl = slice(nq * 512, (nq + 1) * 512)
                            nc.scalar.activation(
                                out=ptA[:, sl], in_=scA[:, sl], func=AF.Exp
                            )
                            nc.vector.tensor_scalar(
                                out=ptB[:, sl], in0=scB[:, sl], scalar1=EA,
                                scalar2=EB, op0=ALU.mult, op1=ALU.add,
                            )
                    pts_cur.append((ptA[:, :], ptB[:, :].bitcast(BF16)))

                    # av of previous pair: group g = st//2 on even slots;
                    # group copies trail two slots behind
                    if p >= 1:
                        if st_ % 2 == 0:
                            emit_av_group_mms(p - 1, st_ // 2, pts_prev)
                        if st_ in (3, 5, 7):
                            g = (st_ - 3) // 2
                            emit_av_group_copy(
                                p - 1, g, "act" if g % 2 == 0 else "dve"
                            )

                    # next pair's qk matmuls fill slots 4-7 (no av there)
                    if p <= 2:
                        pn = p + 1
                        if st_ == 4:
                            qk_next_ps = qk_tile(f"qps{pn}")
                            emit_qk_mms(pn, qk_next_ps, 0, nqs=(0,))
                        if st_ == 5:
                            emit_qk_mms(pn, qk_next_ps, 0, nqs=(1,))
                            qp_next = emit_q_copy(pn, qk_next_ps)
                        if st_ == 6:
                            kq_next_ps = qk_tile(f"kps{pn}")
                            emit_qk_mms(pn, kq_next_ps, 1, nqs=(0,))
                        if st_ == 7:
                            emit_qk_mms(pn, kq_next_ps, 1, nqs=(1,))
                            kp_next = emit_k_copy(pn, kq_next_ps)

                # trailing av copy of pair p-1 (group 3)
                if p >= 1:
                    emit_av_group_copy(p - 1, 3, "dve")
                if p <= 2:
                    qp_s, kp_s = qp_next, kp_next
                pts_prev = pts_cur

            # ---------------- tail: av(3) + normalize + projection --------
            a_all = [
                aallp.tile([128, T], BF16, tag="aall", name=f"aall{ct}")
                for ct in range(NCT)
            ]

            def emit_zb(h_):
                """zb[64, T] psum = broadcast of invz row h_ via a K=4
                one-hot fp32r matmul (e8 row h_%4 selects the head)."""
                zb = qk_tile(f"zb{h_}")
                izv = invzA if h_ < 4 else invzB
                for nq in range(2):
                    nc.tensor.matmul(
                        zb[0:CH, nq * 512:(nq + 1) * 512],
                        e8[:, h_ * CH:(h_ + 1) * CH],
                        izv[:, nq * 512:(nq + 1) * 512],
                        start=True, stop=True,
                    )
                return zb

            def emit_norm_mul(h_, zb):
                """a_all[h_//2] rows = aunbig[:, h_, :] * zb (DVE)."""
                with nc.allow_low_precision(reason="bf16 matmul operands"):
                    if h_ % 2 == 0:
                        nc.vector.tensor_mul(
                            out=a_all[h_ // 2][0:CH, :],
                            in0=aunbig[0:CH, h_, :], in1=zb[0:CH, :],
                        )
                    else:
                        atmp = tmpp.tile([CH, T], BF16, tag="atmp",
                                         name=f"atmp{h_}")
                        nc.vector.tensor_mul(
                            out=atmp[:, :],
                            in0=aunbig[0:CH, h_, :], in1=zb[0:CH, :],
                        )
                        nc.sync.dma_start(
                            out=a_all[h_ // 2][CH:2 * CH, :], in_=atmp[:, :]
                        )

            def emit_proj(m, pps, cks):
                for nq in range(2):
                    dst = pps[:, nq * 512:(nq + 1) * 512]
                    for ck in cks:
                        nc.tensor.matmul(
                            dst,
                            wp_t[ck][:, m * 128:(m + 1) * 128],
                            a_all[ck][:, nq * 512:(nq + 1) * 512],
                            start=(ck == 0), stop=(ck == NCT - 1),
                        )

            # normalize batch A (heads 0-3, in aunbig since pair 2 end)
            nc.sync.dma_start(out=zallA[:, :], in_=aunbig[CH:CH + 1, 0:4, :])
            nc.vector.reciprocal_approx_accurate(
                out=invzfA[:, :], in_=zallA[:, :], scratch=zscr[:, :]
            )
            with nc.allow_low_precision(reason="fp32r matmul operand"):
                nc.vector.tensor_copy(out=invzA[:, :], in_=invzfA[:, :])
            # heads 4,5 Z rows (copied to aunbig at pair-3 end); rows 2,3
            # still hold the 1.0 prefill so the full-tile recip is legal
            nc.sync.dma_start(
                out=zallB[0:2, :], in_=aunbig[CH:CH + 1, 4:6, :]
            )
            nc.vector.reciprocal_approx_accurate(
                out=invzfB[:, :], in_=zallB[:, :], scratch=zscr[:, :]
            )
            with nc.allow_low_precision(reason="fp32r matmul operand"):
                nc.vector.tensor_copy(out=invzB[:, :], in_=invzfB[:, :])

            # interleave: av(3) groups | zb/mul chain | early projection
            zb0 = emit_zb(0)
            emit_av_group_mms(3, 0, pts_prev)
            emit_norm_mul(0, zb0)
            zb1 = emit_zb(1)
            emit_av_group_mms(3, 1, pts_prev)
            emit_norm_mul(1, zb1)
            emit_av_group_copy(3, 0, "act")
            zb2 = emit_zb(2)
            emit_av_group_mms(3, 2, pts_prev)
            emit_norm_mul(2, zb2)
            zb3 = emit_zb(3)
            emit_norm_mul(3, zb3)
            emit_av_group_copy(3, 1, "act")

            # early projection for m=0,1 over ck 0,1 (a_all[0,1] ready)
            pps01 = [sc_tile(f"pps{m}") for m in range(2)]
            for m in range(2):
                emit_proj(m, pps01[m], [0, 1])

            emit_av_group_mms(3, 3, pts_prev)
            emit_av_group_copy(3, 2, "act")
            emit_av_group_copy(3, 3, "dve")

            # heads 4,5 can normalize early (invzB rows 0,1 already valid)
            zb4 = emit_zb(4)
            emit_norm_mul(4, zb4)
            zb5 = emit_zb(5)
            emit_norm_mul(5, zb5)
            for m in range(2):
                emit_proj(m, pps01[m], [2])
            # heads 6,7: re-run the recip with their real Z rows in place
            nc.sync.dma_start(
                out=zallB[2:4, :], in_=aunbig[CH:CH + 1, 6:8, :]
            )
            nc.vector.reciprocal_approx_accurate(
                out=invzfB[:, :], in_=zallB[:, :], scratch=zscr[:, :]
            )
            with nc.allow_low_precision(reason="fp32r matmul operand"):
                nc.vector.tensor_copy(out=invzB[:, :], in_=invzfB[:, :])
            zb6 = emit_zb(6)
            emit_norm_mul(6, zb6)
            zb7 = emit_zb(7)
            emit_norm_mul(7, zb7)

            # finish projection + residual + store
            for m in range(2):
                emit_proj(m, pps01[m], [3])
                nc.vector.tensor_add(
                    out=x_t[m][:, :], in0=pps01[m][:, :], in1=x_t[m][:, :]
                )
                nc.sync.dma_start(
                    out=y_d.ap()[m * 128:(m + 1) * 128, :], in_=x_t[m][:, :]
                )
            for m in range(2, 4):
                pps = sc_tile(f"pps{m}")
                emit_proj(m, pps, [0, 1, 2, 3])
                nc.vector.tensor_add(
                    out=x_t[m][:, :], in0=pps[:, :], in1=x_t[m][:, :]
                )
                nc.sync.dma_start(
                    out=y_d.ap()[m * 128:(m + 1) * 128, :], in_=x_t[m][:, :]
                )

    nc.compile()
    return nc


def make_in_maps(x, gn_weight, gn_bias, w_qkv, b_qkv, w_proj, b_proj):
    import ml_dtypes

    x = np.asarray(x, dtype=np.float32)
    w_qkv = np.asarray(w_qkv, dtype=np.float32)
    b_qkv = np.asarray(b_qkv, dtype=np.float32)
    scale = np.float32(SCALE)
    wq = w_qkv.copy()
    bq = b_qkv.copy()
    for hd in range(NH):
        sl = slice(3 * CH * hd, 3 * CH * hd + 2 * CH)  # q,k rows of this head
        wq[sl] *= scale
        bq[sl] *= scale
    # Column order expected by the kernel: per head-pair p the contiguous
    # blocks [q(2p) | q(2p+1) | k(2p) | k(2p+1)] (256 cols each), then all
    # v blocks (with per-head Z columns).
    perm = []
    for p in range(NH // 2):
        for hd in (2 * p, 2 * p + 1):
            perm.extend(range(3 * CH * hd, 3 * CH * hd + CH))          # q
        for hd in (2 * p, 2 * p + 1):
            perm.extend(range(3 * CH * hd + CH, 3 * CH * hd + 2 * CH))  # k
    for hd in range(NH):
        perm.extend(range(3 * CH * hd + 2 * CH, 3 * CH * hd + 3 * CH))  # v
    perm = np.asarray(perm)
    wq = wq[perm]
    bq = bq[perm]
    # v-section gains a zero-weight column per head whose bias is 1.0 (the
    # Z column of v^T); qk section stays 1024 wide.
    wq2 = np.zeros((C, 2 * C + VW), np.float32)
    vb = np.zeros(VW, np.float32)
    wq2[:, 0:2 * C] = wq.T[:, 0:2 * C]
    for hd in range(NH):
        wq2[:, 2 * C + 65 * hd:2 * C + 65 * hd + CH] = \
            wq.T[:, 2 * C + CH * hd:2 * C + CH * (hd + 1)]
        vb[65 * hd:65 * hd + CH] = bq[2 * C + CH * hd:2 * C + CH * (hd + 1)]
        vb[65 * hd + CH] = 1.0
    wqkvT = np.ascontiguousarray(wq2.astype(ml_dtypes.bfloat16))
    wprojT = np.ascontiguousarray(
        np.asarray(w_proj, np.float32).T.astype(ml_dtypes.bfloat16)
    )
    vrow = np.concatenate(
        [vb, np.ones(512, np.float32)]
    ).reshape(1, -1).astype(ml_dtypes.bfloat16)

    gamma = np.asarray(gn_weight, np.float32).reshape(NCT, 128).T
    beta = np.asarray(gn_bias, np.float32).reshape(NCT, 128).T
    gred = np.zeros((128, 8), np.float32)
    gbcast = np.zeros((8, 128), np.float32)
    for c in range(128):
        gred[c, c // 16] = 1.0 / 16.0
        gbcast[c // 16, c] = 1.0
    e8 = np.zeros((4, NH * CH), np.float32)
    for g in range(8):
        e8[g % 4, g * CH:(g + 1) * CH] = 1.0
    cblob = np.ascontiguousarray(np.concatenate([gamma, beta, gred], axis=1))
    # bias columns: per pair p the q col then k col, then bproj columns
    bcols = np.zeros((128, 12), np.float32)
    for p in range(NH // 2):
        bcols[:, 2 * p] = bq[256 * p:256 * p + 128]
        bcols[:, 2 * p + 1] = bq[256 * p + 128:256 * p + 256]
    bcols[:, 8:12] = np.asarray(b_proj, np.float32).reshape(NCT, 128).T

    common = dict(
        wqkvT=wqkvT, wprojT=wprojT, cblob=cblob, bcols=bcols,
        vrow=np.ascontiguousarray(vrow), gbcast=gbcast, e8=e8,
    )
    in_maps = []
    for b in range(B):
        m = dict(common)
        m["x"] = np.ascontiguousarray(x[b].reshape(C, T))
        in_maps.append(m)
    return in_maps


def kernel(x, gn_weight, gn_bias, w_qkv, b_qkv, w_proj, b_proj, _trace=False):
    if "nc" not in _CACHE:
        _CACHE["nc"] = build_kernel()
    nc = _CACHE["nc"]
    in_maps = make_in_maps(x, gn_weight, gn_bias, w_qkv, b_qkv, w_proj, b_proj)
    res = bass_utils.run_bass_kernel_spmd(
        nc, in_maps, core_ids=list(range(B)), trace=_trace
    )
    out = np.stack([r["y"].reshape(C, HH, WW) for r in res.results], axis=0)
    if _trace:
        _CACHE["last_result"] = res
    return out
